# revision 2
# baseline (speedup 1.0000x reference)
"""Trainium2 Bass kernel for nn_NumDualDescriptorAB.

Reference computation:
    agg[b,w]   = mean(seq[b, w:w+8, :], axis=0)          (sliding window, Nw = S-7)
    y[b,w]     = agg[b,w] @ M.T
    Nk[w]      = Acoeff[:, w%L] * Bbasis[w%L, :]
    D          = mean((y - Nk)^2)

Algebraic decomposition (device computes only the quadratic term):
    count = B*Nw*m
    t1 = <M^T M, G>_F   with G = sum_{b,w} agg^T agg    (m x m)
    t2 = sum_s seqsum[s] . P[s]   (linear in seq -> exact host reduction)
    t3 = B * ||Nk||^2
    D  = (t1 - 2 t2 + t3) / count

Device schedule (v2 -- see git history for the 121-window variant):
  - seq rows are chunked by 128 (16 chunks exactly cover S=2048).  Window
    chunk c (windows 128c..128c+127) contracts seq row-chunks c and c+1 in
    ONE DoubleRow fp8 matmul (256-deep contraction, banded weights wdr).
    The last chunk (121 windows) is a normal matmul with wtail.
  - agg chunks (f32 PSUM) are cast to fp8 in GROUPS of 3 PSUM banks with a
    single DVE CAST + single ACT copy per group: the ~125/~300-cycle fixed
    instruction overheads amortize 3x (per-chunk cast cost ~300ns vs ~440
    for per-chunk casts; the casts, not the PE, are the steady-state wall).
  - Gram: consecutive agg chunk pairs feed DoubleRow fp8 matmuls (256-deep
    contraction over windows), 8 pairs x 4 batches = 32 matmuls accumulate
    G in one PSUM bank.
  - PE warmup: the HAM clock gate tracks MAC *utilization*, not occupancy
    (1-col dummies earn no credit -- measured).  Full-width 512-col dummy
    matmuls with no data deps run from engine start, so the PE unthrottles
    (1.2 -> 2.4 GHz) ~2.8us after body start instead of ~3us after the
    first real window matmul.
  - End pads: a few more wide dummies after the last gram keep the PE's
    trailing-utilization window hot, so the runtime-injected per-semaphore
    teardown clears on the Tensor queue (51 of them, dispatch-bound at the
    PE clock) start at 2.4 GHz (~57ns each) instead of 1.2 (~115ns).
  - Piece-0 DMA (wdr|wtail|chunks 0-1) issues from the Scalar queue, whose
    engine body starts ~0.9us before Sync's; remaining pieces go on the
    Sync queue in consumption order.
  - The G write-back is a fire-and-forget single-packet DMA issued after
    the TileContext (dead-semaphore then_inc); its transfer hides in the
    runtime teardown.

Host side (float64): P/seqsum/t2, t3, M^T M, and the final combine.
"""

import os

# The device run goes through jax's axon/neuron backend; a cpu-only pin
# (used for reference computations elsewhere) would hide the NeuronCores.
if os.environ.get("JAX_PLATFORMS", "").strip() == "cpu":
    del os.environ["JAX_PLATFORMS"]

import numpy as np
import ml_dtypes

B, S, m, L, RANK = 32, 2048, 128, 64, 8
Nw = S - RANK + 1  # 2041
NCORES = 8
BPC = B // NCORES  # batches per core = 4
NCH = S // 128  # 16 row chunks of 128
TAILW = Nw - (NCH - 1) * 128  # 121 windows in the last chunk
CW = BPC * m  # free columns per chunk = 512
WCOLS = 3 * m  # wdr (2*128) + wtail (128), stored ahead of seq data

FP8 = ml_dtypes.float8_e4m3

_NC_CACHE = {}

N_WARM = 5   # full-width dummy matmuls at body start (HAM util credit)
N_PAD = 5    # full-width dummies after the last gram (keep clock for clears)
DVE_X = 248  # cast split: DVE takes cols [0, DVE_X), ACT the rest
# seq DMA pieces as chunk ranges.  Piece 0 additionally carries wdr+wtail
# in the same dma_start and is issued from the Scalar queue (earliest
# engine body start); the rest go on the Sync queue in consumption order.
PIECE_CHUNKS = [2, 4, 4, 3, 3]


def _build_nc():
    import concourse.bacc as bacc
    import concourse.mybir as mybir
    import concourse.tile as tile

    f8 = mybir.dt.float8e4
    f32 = mybir.dt.float32
    DR = mybir.MatmulPerfMode.DoubleRow

    nc = bacc.Bacc("TRN2", target_bir_lowering=False, debug=False,
                   enable_partition_id=False)

    seq_d = nc.dram_tensor("seq", [128, WCOLS + NCH * CW], f8,
                           kind="ExternalInput")
    out_d = nc.dram_tensor("out", [128, m], f32, kind="ExternalOutput")

    # raw (non-tile) SBUF tensor so the fire-and-forget DMA below has a
    # concrete access pattern
    s_out = nc.alloc_sbuf_tensor("s_out", [128, m], f32)

    with tile.TileContext(nc) as tc:
        with (
            tc.tile_pool(name="const", bufs=1) as cpool,
            tc.tile_pool(name="psA", bufs=1, space="PSUM") as poolA,
            tc.tile_pool(name="psB", bufs=1, space="PSUM") as poolB,
            tc.tile_pool(name="psG", bufs=1, space="PSUM") as poolG,
            tc.tile_pool(name="psD", bufs=1, space="PSUM") as poolD,
        ):
            big = cpool.tile([128, WCOLS + NCH * CW], f8, tag="big")
            aggb = cpool.tile([128, NCH * CW], f8, tag="aggb")
            dum = cpool.tile([128, CW], f8, tag="dum")

            wdr = big[:, 0:2 * m].rearrange("p (i w) -> p i w", w=m)
            wtl = big[:, 2 * m:3 * m]
            seqv = big[:, WCOLS:].rearrange("p (c n) -> p c n", n=CW)
            aggv = aggb[:].rearrange("p (c n) -> p c n", n=CW)

            pA = poolA.tile([128, 3, CW], f32, tag="A")
            pB = poolB.tile([128, 3, CW], f32, tag="B")
            G_ps = poolG.tile([128, m], f32, tag="G")
            dscr = poolD.tile([128, CW], f32, tag="D")

            def grp(c):
                g = c // 3
                return (pA if g % 2 == 0 else pB), g, c % 3

            # dummy source: DVE memset is cheap and Vector is idle early
            nc.vector.memset(dum[:], 0)

            # --- DMA issue: piece 0 from Scalar (earliest engine body),
            # the rest from Sync, all in consumption order.
            a = 0
            for pc, n in enumerate(PIECE_CHUNKS):
                if pc == 0:
                    nc.scalar.dma_start(out=big[:, 0:WCOLS + n * CW],
                                        in_=seq_d[:, 0:WCOLS + n * CW])
                else:
                    lo, hi = WCOLS + a * CW, WCOLS + (a + n) * CW
                    nc.sync.dma_start(out=big[:, lo:hi], in_=seq_d[:, lo:hi])
                a += n

            # Early no-dep ACT op: forces the ~1.3us ACT_TABLE_LOAD (which
            # tile otherwise schedules behind the first cast's semaphore
            # wait) to run during the DMA fill phase.
            warm = cpool.tile([128, 1], f8, tag="warm")
            nc.scalar.copy(warm[:], dum[:, 0:1])

            # --- PE warmup: full-width, no data deps, garbage-in-garbage-
            # out into a scratch PSUM bank nothing reads.
            for _ in range(N_WARM):
                nc.tensor.matmul(dscr[:], dum[:, 0:m], dum[:],
                                 start=True, stop=True, skip_group_check=True)

            # --- main pipeline ---
            def emit_win(c):
                P, g, k = grp(c)
                if c < NCH - 1:
                    nc.tensor.matmul(P[:, k, :], wdr, seqv[:, c:c + 2, :],
                                     start=True, stop=True, perf_mode=DR)
                else:
                    nc.tensor.matmul(P[:, k, :], wtl, seqv[:, c, :],
                                     start=True, stop=True)

            def emit_cast(g):
                # chunks 3g .. min(3g+3, NCH)-1 as one grouped cast per engine
                n = min(3, NCH - 3 * g)
                P = pA if g % 2 == 0 else pB
                nc.vector.tensor_copy(aggv[:, 3 * g:3 * g + n, 0:DVE_X],
                                      P[:, 0:n, 0:DVE_X])
                nc.scalar.copy(aggv[:, 3 * g:3 * g + n, DVE_X:CW],
                               P[:, 0:n, DVE_X:CW])

            def emit_gram(p):
                for j in range(BPC):
                    blk = aggv[:, 2 * p:2 * p + 2, j * m:(j + 1) * m]
                    nc.tensor.matmul(
                        G_ps[:], blk, blk,
                        start=(p == 0 and j == 0),
                        stop=(p == NCH // 2 - 1 and j == BPC - 1),
                        perf_mode=DR, skip_group_check=True,
                    )

            # pair p (chunks 2p, 2p+1) is ready after cast group g when
            # 2p+1 <= 3g+2  ->  emit grams per completed cast group.
            NG = (NCH + 2) // 3  # 6 cast groups (last has 1 chunk)
            done_pairs = 0
            for g in range(NG):
                hi = min(3 * g + 3, NCH)
                for c in range(3 * g, hi):
                    emit_win(c)
                emit_cast(g)
                while done_pairs < NCH // 2 and 2 * done_pairs + 1 < hi:
                    emit_gram(done_pairs)
                    done_pairs += 1

            nc.vector.tensor_copy(s_out.ap(), G_ps[:])

            # --- end pads: keep the PE's trailing HAM utilization window
            # hot into the teardown semaphore clears.  Reading aggb's last
            # chunk anchors them after the final cast in the PE stream.
            for _ in range(N_PAD):
                nc.tensor.matmul(dscr[:], aggv[:, NCH - 1, 0:m],
                                 aggv[:, NCH - 1, :],
                                 start=True, stop=True, skip_group_check=True)

    # Fire-and-forget output DMA (walrus requires sync info on DGE ops, so
    # give it a completion semaphore nothing waits on).  The HBM write
    # receipt overlaps the runtime teardown.
    ff_sem = nc.alloc_semaphore("ff_out")
    nc.sync.dma_start(out=out_d[:], in_=s_out.ap(),
                      single_packet=True).then_inc(ff_sem, 16)

    nc.compile()
    return nc


def get_nc():
    if "nc" not in _NC_CACHE:
        _NC_CACHE["nc"] = _build_nc()
    return _NC_CACHE["nc"]


def host_prep(seq_batch, M, Acoeff, Bbasis):
    """Build per-core device inputs + host-side exact terms."""
    # seq image: [128, NCH, BPC, m] with img[p, c, j] = seq[4k+j, 128c+p]
    g = np.asarray(seq_batch, np.float32).astype(FP8)  # [B, S, m]
    imgs = np.ascontiguousarray(
        g.reshape(NCORES, BPC, NCH, 128, m).transpose(0, 3, 2, 1, 4)
    ).reshape(NCORES, 128, NCH * CW)

    # DoubleRow banded window weights: out window w (0..127) contracts
    # k-tile i, row r where 128i + r - w in [0, 8).
    r = np.arange(128)[:, None]
    w = np.arange(128)[None, :]
    wk0 = (((r - w) >= 0) & ((r - w) < RANK)).astype(np.float32) / RANK
    wk1 = (((128 + r - w) >= 0) & ((128 + r - w) < RANK)).astype(np.float32) / RANK
    wtail = wk0 * (w < TAILW)
    wmat = np.concatenate([wk0, wk1, wtail], axis=1).astype(FP8)  # [128, 384]

    full = np.concatenate(
        [np.broadcast_to(wmat, (NCORES, 128, WCOLS)), imgs], axis=2)
    full = np.ascontiguousarray(full)

    # linear terms in float64 on host: t2 = <seqsum, P>, t3 = B*||Nk||^2
    M64 = np.asarray(M, np.float64)
    kmod = np.arange(Nw) % L
    Nk = (np.asarray(Acoeff, np.float64).T[kmod]
          * np.asarray(Bbasis, np.float64)[kmod])  # [Nw, m]
    Ntil = Nk @ M64  # [Nw, m]
    csum = np.concatenate([np.zeros((1, m)), np.cumsum(Ntil, axis=0)])
    s = np.arange(S)
    lo = np.maximum(s - (RANK - 1), 0)
    hi = np.minimum(s, Nw - 1)
    P = (csum[hi + 1] - csum[lo]) / RANK  # [S, m]

    seqsum = np.asarray(seq_batch, np.float64).sum(axis=0)  # [S, m]
    t2 = float((seqsum * P).sum())
    t3 = B * float((Nk ** 2).sum())
    MtM = M64.T @ M64
    return full, MtM, t2, t3


def combine(results, MtM, t2, t3):
    """results: list of 8 arrays [128, 128] f32 (per-core G) -> scalar D."""
    G = np.zeros((m, m), np.float64)
    for r in results:
        G += np.asarray(r, np.float64)
    t1 = float((MtM * G).sum())
    D = (t1 - 2.0 * t2 + t3) / (B * Nw * m)
    return np.float32(D)


def kernel(seq_batch, M, Acoeff, Bbasis):
    from concourse.bass_utils import run_bass_kernel_spmd

    seq_batch = np.asarray(seq_batch, np.float32)
    full, MtM, t2, t3 = host_prep(seq_batch, M, Acoeff, Bbasis)

    nc = get_nc()
    in_maps = [{"seq": full[c]} for c in range(NCORES)]
    res = run_bass_kernel_spmd(nc, in_maps, core_ids=list(range(NCORES)))
    outs = [res.results[c]["out"] for c in range(NCORES)]
    return combine(outs, MtM, t2, t3)


# revision 3
# speedup vs baseline: 1.1447x; 1.1447x over previous
"""Trainium2 Bass kernel for nn_NumDualDescriptorAB.

Reference computation:
    agg[b,w]   = mean(seq[b, w:w+8, :], axis=0)          (sliding window, Nw = S-7)
    y[b,w]     = agg[b,w] @ M.T
    Nk[w]      = Acoeff[:, w%L] * Bbasis[w%L, :]
    D          = mean((y - Nk)^2)

Algebraic decomposition (device computes only the quadratic term):
    count = B*Nw*m
    t1 = <M^T M, G>_F   with G = sum_{b,w} agg^T agg    (m x m)
    t2 = sum_s seqsum[s] . P[s]   (linear in seq -> exact host reduction)
    t3 = B * ||Nk||^2
    D  = (t1 - 2 t2 + t3) / count

Device schedule (v3):
  - seq rows are chunked by 128 (16 chunks exactly cover S=2048).  Window
    chunk c (windows 128c..128c+127) contracts seq row-chunks c and c+1 in
    ONE DoubleRow fp8 matmul (256-deep contraction, banded weights wdr);
    the last chunk (121 windows) is a normal matmul with wtail.
  - agg chunks (f32 PSUM) are cast to fp8 whole-chunk, ALTERNATING engines
    (DVE takes even chunks, ACT odd): one 512-col instruction per chunk
    amortizes the fixed instruction overhead (~335ns/chunk wall vs ~440
    for per-chunk column-split casts).
  - Gram: consecutive agg chunk pairs feed DoubleRow fp8 matmuls (256-deep
    contraction over windows), 8 pairs x 4 batches = 32 matmuls accumulate
    G in one PSUM bank.  PE steady state ~372ns/chunk (win 216 + grams).
  - PE warmup: the HAM clock gate tracks MAC *utilization*; zero operands
    are clock-gated and earn NO credit (measured).  Wide 512-col dummies
    on NONZERO data (dum memset to 1.0) run from engine body start so the
    PE unthrottles (1.2 -> 2.4 GHz) around when real data lands.
  - All seq DMA pieces go on the single Sync HWDGE queue in consumption
    order: queue-level serialization gives piece 0 the full bandwidth
    (a second queue's pieces otherwise delay piece 0's completion by ~1us
    -- measured).  Scalar stays free: its ACT_TABLE_LOAD (1.3us, engine-
    blocking) must finish before the first odd-chunk cast.
  - Runtime overheads outside kernel control (~10.6us of the measured
    window): gauge's clock starts at the framework's GpSimd const-memsets
    (~1.4us before the post-barrier engine body start), and the teardown
    runs ~51 serial semaphore clears on the Tensor queue at a fixed
    ~115ns dispatch (5.9us) behind a ~2.5us barrier+ring.

Host side (float64): P/seqsum/t2, t3, M^T M, and the final combine.
"""

import os

# The device run goes through jax's axon/neuron backend; a cpu-only pin
# (used for reference computations elsewhere) would hide the NeuronCores.
if os.environ.get("JAX_PLATFORMS", "").strip() == "cpu":
    del os.environ["JAX_PLATFORMS"]

import numpy as np
import ml_dtypes

B, S, m, L, RANK = 32, 2048, 128, 64, 8
Nw = S - RANK + 1  # 2041
NCORES = 8
BPC = B // NCORES  # batches per core = 4
NCH = S // 128  # 16 row chunks of 128
TAILW = Nw - (NCH - 1) * 128  # 121 windows in the last chunk
CW = BPC * m  # free columns per chunk = 512
WCOLS = 3 * m  # wdr (2*128) + wtail (128), stored ahead of seq data

FP8 = ml_dtypes.float8_e4m3

_NC_CACHE = {}

N_WARM = 5   # full-width nonzero dummy matmuls at body start (HAM credit)
NPRE = 4     # window-matmul prefill depth (PSUM pool is 6 deep)
PIECE_CHUNKS = [2, 4, 4, 3, 3]  # seq DMA pieces (chunk counts)


def _build_nc():
    import concourse.bacc as bacc
    import concourse.mybir as mybir
    import concourse.tile as tile

    f8 = mybir.dt.float8e4
    f32 = mybir.dt.float32
    DR = mybir.MatmulPerfMode.DoubleRow

    nc = bacc.Bacc("TRN2", target_bir_lowering=False, debug=False,
                   enable_partition_id=False)

    seq_d = nc.dram_tensor("seq", [128, WCOLS + NCH * CW], f8,
                           kind="ExternalInput")
    out_d = nc.dram_tensor("out", [128, m], f32, kind="ExternalOutput")

    # raw (non-tile) SBUF tensor so the fire-and-forget DMA below has a
    # concrete access pattern
    s_out = nc.alloc_sbuf_tensor("s_out", [128, m], f32)

    with tile.TileContext(nc) as tc:
        with (
            tc.tile_pool(name="const", bufs=1) as cpool,
            tc.tile_pool(name="psa", bufs=6, space="PSUM") as pspool,
            tc.tile_pool(name="psacc", bufs=1, space="PSUM") as accpool,
        ):
            big = cpool.tile([128, WCOLS + NCH * CW], f8, tag="big")
            aggb = cpool.tile([128, NCH * CW], f8, tag="aggb")
            dum = cpool.tile([128, CW], f8, tag="dum")

            wdr = big[:, 0:2 * m].rearrange("p (i w) -> p i w", w=m)
            wtl = big[:, 2 * m:3 * m]
            seqv = big[:, WCOLS:].rearrange("p (c n) -> p c n", n=CW)
            aggv = aggb[:].rearrange("p (c n) -> p c n", n=CW)

            G_ps = accpool.tile([128, m], f32, tag="G")
            dscr = accpool.tile([128, CW], f32, tag="scr")

            # nonzero dummy source (zeros are clock-gated in the PE and
            # earn no HAM utilization credit)
            nc.vector.memset(dum[:], 1.0)

            # --- DMA issue (t=0): all pieces on the sync HWDGE ring,
            # serial, so arrival order == consumption order and piece 0
            # gets the full bandwidth.
            a = 0
            for n in PIECE_CHUNKS:
                lo = 0 if a == 0 else WCOLS + a * CW
                hi = WCOLS + (a + n) * CW
                nc.sync.dma_start(out=big[:, lo:hi], in_=seq_d[:, lo:hi])
                a += n

            # Early no-dep ACT op: pulls the ~1.3us engine-blocking
            # ACT_TABLE_LOAD into the DMA fill phase.
            warm = cpool.tile([128, 1], f8, tag="warm")
            nc.scalar.copy(warm[:], dum[:, 0:1])

            # --- PE warmup: full-width, nonzero, no data deps beyond the
            # memset; garbage results go to a scratch PSUM bank.
            for _ in range(N_WARM):
                nc.tensor.matmul(dscr[:], dum[:, 0:m], dum[:],
                                 start=True, stop=True, skip_group_check=True)

            # --- main pipeline ---
            agg_tiles = {}

            def emit_win(c):
                agg_ps = pspool.tile([128, CW], f32, tag="aggps",
                                     name=f"agg{c}")
                agg_tiles[c] = agg_ps
                if c < NCH - 1:
                    nc.tensor.matmul(agg_ps[:], wdr, seqv[:, c:c + 2, :],
                                     start=True, stop=True, perf_mode=DR)
                else:
                    nc.tensor.matmul(agg_ps[:], wtl, seqv[:, c, :],
                                     start=True, stop=True)

            def emit_gram(p):
                for j in range(BPC):
                    blk = aggv[:, 2 * p:2 * p + 2, j * m:(j + 1) * m]
                    nc.tensor.matmul(
                        G_ps[:], blk, blk,
                        start=(p == 0 and j == 0),
                        stop=(p == NCH // 2 - 1 and j == BPC - 1),
                        perf_mode=DR, skip_group_check=True,
                    )

            for c in range(NPRE):
                emit_win(c)
            for c in range(NCH):
                # whole-chunk cast, alternating engines
                if c % 2 == 0:
                    nc.vector.tensor_copy(aggv[:, c, :], agg_tiles[c][:])
                else:
                    nc.scalar.copy(aggv[:, c, :], agg_tiles[c][:])
                if c % 2 == 1:
                    emit_gram(c // 2)
                if c + NPRE < NCH:
                    emit_win(c + NPRE)

            nc.vector.tensor_copy(s_out.ap(), G_ps[:])

    # Fire-and-forget output DMA (walrus requires sync info on DGE ops, so
    # give it a completion semaphore nothing waits on).  The HBM write
    # receipt overlaps the runtime teardown.
    ff_sem = nc.alloc_semaphore("ff_out")
    nc.sync.dma_start(out=out_d[:], in_=s_out.ap(),
                      single_packet=True).then_inc(ff_sem, 16)

    nc.compile()
    return nc


def get_nc():
    if "nc" not in _NC_CACHE:
        _NC_CACHE["nc"] = _build_nc()
    return _NC_CACHE["nc"]


def host_prep(seq_batch, M, Acoeff, Bbasis):
    """Build per-core device inputs + host-side exact terms."""
    # seq image: [128, NCH, BPC, m] with img[p, c, j] = seq[4k+j, 128c+p]
    g = np.asarray(seq_batch, np.float32).astype(FP8)  # [B, S, m]
    imgs = np.ascontiguousarray(
        g.reshape(NCORES, BPC, NCH, 128, m).transpose(0, 3, 2, 1, 4)
    ).reshape(NCORES, 128, NCH * CW)

    # DoubleRow banded window weights: out window w (0..127) contracts
    # k-tile i, row r where 128i + r - w in [0, 8).
    r = np.arange(128)[:, None]
    w = np.arange(128)[None, :]
    wk0 = (((r - w) >= 0) & ((r - w) < RANK)).astype(np.float32) / RANK
    wk1 = (((128 + r - w) >= 0) & ((128 + r - w) < RANK)).astype(np.float32) / RANK
    wtail = wk0 * (w < TAILW)
    wmat = np.concatenate([wk0, wk1, wtail], axis=1).astype(FP8)  # [128, 384]

    full = np.concatenate(
        [np.broadcast_to(wmat, (NCORES, 128, WCOLS)), imgs], axis=2)
    full = np.ascontiguousarray(full)

    # linear terms in float64 on host: t2 = <seqsum, P>, t3 = B*||Nk||^2
    M64 = np.asarray(M, np.float64)
    kmod = np.arange(Nw) % L
    Nk = (np.asarray(Acoeff, np.float64).T[kmod]
          * np.asarray(Bbasis, np.float64)[kmod])  # [Nw, m]
    Ntil = Nk @ M64  # [Nw, m]
    csum = np.concatenate([np.zeros((1, m)), np.cumsum(Ntil, axis=0)])
    s = np.arange(S)
    lo = np.maximum(s - (RANK - 1), 0)
    hi = np.minimum(s, Nw - 1)
    P = (csum[hi + 1] - csum[lo]) / RANK  # [S, m]

    seqsum = np.asarray(seq_batch, np.float64).sum(axis=0)  # [S, m]
    t2 = float((seqsum * P).sum())
    t3 = B * float((Nk ** 2).sum())
    MtM = M64.T @ M64
    return full, MtM, t2, t3


def combine(results, MtM, t2, t3):
    """results: list of 8 arrays [128, 128] f32 (per-core G) -> scalar D."""
    G = np.zeros((m, m), np.float64)
    for r in results:
        G += np.asarray(r, np.float64)
    t1 = float((MtM * G).sum())
    D = (t1 - 2.0 * t2 + t3) / (B * Nw * m)
    return np.float32(D)


def kernel(seq_batch, M, Acoeff, Bbasis):
    from concourse.bass_utils import run_bass_kernel_spmd

    seq_batch = np.asarray(seq_batch, np.float32)
    full, MtM, t2, t3 = host_prep(seq_batch, M, Acoeff, Bbasis)

    nc = get_nc()
    in_maps = [{"seq": full[c]} for c in range(NCORES)]
    res = run_bass_kernel_spmd(nc, in_maps, core_ids=list(range(NCORES)))
    outs = [res.results[c]["out"] for c in range(NCORES)]
    return combine(outs, MtM, t2, t3)


# revision 7
# speedup vs baseline: 1.2106x; 1.0576x over previous
"""Trainium2 Bass kernel for nn_NumDualDescriptorAB.

Reference computation:
    agg[b,w]   = mean(seq[b, w:w+8, :], axis=0)          (sliding window, Nw = S-7)
    y[b,w]     = agg[b,w] @ M.T
    Nk[w]      = Acoeff[:, w%L] * Bbasis[w%L, :]
    D          = mean((y - Nk)^2)

Algebraic decomposition (device computes only the quadratic term):
    count = B*Nw*m
    t1 = <M^T M, G>_F   with G = sum_{b,w} agg^T agg    (m x m)
    t2 = sum_s seqsum[s] . P[s]   (linear in seq -> exact host reduction)
    t3 = B * ||Nk||^2
    D  = (t1 - 2 t2 + t3) / count

Device schedule (v3):
  - seq rows are chunked by 128 (16 chunks exactly cover S=2048).  Window
    chunk c (windows 128c..128c+127) contracts seq row-chunks c and c+1 in
    ONE DoubleRow fp8 matmul (256-deep contraction, banded weights wdr);
    the last chunk (121 windows) is a normal matmul with wtail.
  - agg chunks (f32 PSUM) are cast to fp8 whole-chunk, ALTERNATING engines
    (DVE takes even chunks, ACT odd): one 512-col instruction per chunk
    amortizes the fixed instruction overhead (~335ns/chunk wall vs ~440
    for per-chunk column-split casts).
  - Gram: consecutive agg chunk pairs feed DoubleRow fp8 matmuls (256-deep
    contraction over windows), 8 pairs x 4 batches = 32 matmuls accumulate
    G in one PSUM bank.  PE steady state ~372ns/chunk (win 216 + grams).
  - PE warmup: the HAM clock gate tracks MAC *utilization*; zero operands
    are clock-gated and earn NO credit (measured).  Wide 512-col dummies
    on NONZERO data (dum memset to 1.0) run from engine body start so the
    PE unthrottles (1.2 -> 2.4 GHz) around when real data lands.
  - All seq DMA pieces go on the single Sync HWDGE queue in consumption
    order: queue-level serialization gives piece 0 the full bandwidth
    (a second queue's pieces otherwise delay piece 0's completion by ~1us
    -- measured).  Scalar stays free: its ACT_TABLE_LOAD (1.3us, engine-
    blocking) must finish before the first odd-chunk cast.
  - Runtime overheads outside kernel control (~10.6us of the measured
    window): gauge's clock starts at the framework's GpSimd const-memsets
    (~1.4us before the post-barrier engine body start), and the teardown
    runs ~51 serial semaphore clears on the Tensor queue at a fixed
    ~115ns dispatch (5.9us) behind a ~2.5us barrier+ring.

Host side (float64): P/seqsum/t2, t3, M^T M, and the final combine.
"""

import os

# The device run goes through jax's axon/neuron backend; a cpu-only pin
# (used for reference computations elsewhere) would hide the NeuronCores.
if os.environ.get("JAX_PLATFORMS", "").strip() == "cpu":
    del os.environ["JAX_PLATFORMS"]

import numpy as np
import ml_dtypes

B, S, m, L, RANK = 32, 2048, 128, 64, 8
Nw = S - RANK + 1  # 2041
NCORES = 8
BPC = B // NCORES  # batches per core = 4
NCH = S // 128  # 16 row chunks of 128
TAILW = Nw - (NCH - 1) * 128  # 121 windows in the last chunk
CW = BPC * m  # free columns per chunk = 512
WCOLS = 3 * m  # wdr (2*128) + wtail (128), stored ahead of seq data

FP8 = ml_dtypes.float8_e4m3

_NC_CACHE = {}

N_WARM = 6   # full-width nonzero dummy matmuls at body start (HAM credit)
NPRE = 4     # window-matmul prefill depth (PSUM pool is 6 deep)
# seq DMA pieces (chunk counts) alternating Sync/Scalar HWDGE queues in
# consumption order.  Two queues double the descriptor-processing rate,
# and smaller pieces reach their 16th (last sub-engine) completion
# increment sooner -- the piece semaphore only fires when the slowest of
# the 16 DMA engines finishes its share.
PIECE_CHUNKS = [2, 2, 4, 4, 2, 2]  # piece 0 also carries wdr+wtail


def _build_nc():
    import concourse.bacc as bacc
    import concourse.mybir as mybir
    import concourse.tile as tile

    f8 = mybir.dt.float8e4
    f32 = mybir.dt.float32
    DR = mybir.MatmulPerfMode.DoubleRow

    nc = bacc.Bacc("TRN2", target_bir_lowering=False, debug=False,
                   enable_partition_id=False)

    seq_d = nc.dram_tensor("seq", [128, WCOLS + NCH * CW], f8,
                           kind="ExternalInput")
    out_d = nc.dram_tensor("out", [128, m], f32, kind="ExternalOutput")

    # raw (non-tile) SBUF tensor so the fire-and-forget DMA below has a
    # concrete access pattern
    s_out = nc.alloc_sbuf_tensor("s_out", [128, m], f32)

    with tile.TileContext(nc) as tc:
        with (
            tc.tile_pool(name="const", bufs=1) as cpool,
            tc.tile_pool(name="psa", bufs=3, space="PSUM") as pspool,
            tc.tile_pool(name="psacc", bufs=1, space="PSUM") as accpool,
        ):
            big = cpool.tile([128, WCOLS + NCH * CW], f8, tag="big")
            aggb = cpool.tile([128, NCH * CW], f8, tag="aggb")
            dum = cpool.tile([128, CW], f8, tag="dum")

            wdr = big[:, 0:2 * m].rearrange("p (i w) -> p i w", w=m)
            wtl = big[:, 2 * m:3 * m]
            seqv = big[:, WCOLS:].rearrange("p (c n) -> p c n", n=CW)
            aggv = aggb[:].rearrange("p (c n) -> p c n", n=CW)

            G_ps = accpool.tile([128, m], f32, tag="G")
            dscr = accpool.tile([128, CW], f32, tag="scr")

            # nonzero dummy source (zeros are clock-gated in the PE and
            # earn no HAM utilization credit)
            nc.vector.memset(dum[:], 1.0)

            # --- DMA issue (t=0): pieces alternate between the two HWDGE
            # rings (Sync, Scalar), both issuing from their body start, so
            # the early pieces' last-straggler completions land sooner.
            a = 0
            for pc, n in enumerate(PIECE_CHUNKS):
                lo = 0 if a == 0 else WCOLS + a * CW
                hi = WCOLS + (a + n) * CW
                eng = nc.sync if pc % 2 == 0 else nc.scalar
                eng.dma_start(out=big[:, lo:hi], in_=seq_d[:, lo:hi])
                a += n

            # Early no-dep ACT op: pulls the ~1.3us engine-blocking
            # ACT_TABLE_LOAD into the DMA fill phase.
            warm = cpool.tile([128, 1], f8, tag="warm")
            nc.scalar.copy(warm[:], dum[:, 0:1])

            # --- PE warmup: full-width, nonzero, no data deps beyond the
            # memset; garbage results go to a scratch PSUM bank.
            for _ in range(N_WARM):
                nc.tensor.matmul(dscr[:], dum[:, 0:m], dum[:],
                                 start=True, stop=True, skip_group_check=True)

            # --- main pipeline ---
            # agg PSUM tiles are allocated per PAIR of chunks (2 adjacent
            # banks) so one cast instruction covers both chunks of a gram
            # pair: the ~125/~300-cycle fixed cast overheads amortize 2x
            # and the pair's grams wait on a single engine's cast.
            pair_tiles = {}

            def emit_win(c):
                p, k = c // 2, c % 2
                if k == 0:
                    pair_tiles[p] = pspool.tile([128, 2, CW], f32,
                                                tag="aggps", name=f"agg{p}")
                agg_ps = pair_tiles[p]
                if c < NCH - 1:
                    nc.tensor.matmul(agg_ps[:, k, :], wdr,
                                     seqv[:, c:c + 2, :],
                                     start=True, stop=True, perf_mode=DR)
                else:
                    nc.tensor.matmul(agg_ps[:, k, :], wtl, seqv[:, c, :],
                                     start=True, stop=True)

            def emit_cast(p):
                dst = aggv[:, 2 * p:2 * p + 2, :]
                if p % 2 == 0:
                    nc.vector.tensor_copy(dst, pair_tiles[p][:])
                else:
                    nc.scalar.copy(dst, pair_tiles[p][:])

            def emit_gram(p):
                for j in range(BPC):
                    blk = aggv[:, 2 * p:2 * p + 2, j * m:(j + 1) * m]
                    nc.tensor.matmul(
                        G_ps[:], blk, blk,
                        start=(p == 0 and j == 0),
                        stop=(p == NCH // 2 - 1 and j == BPC - 1),
                        perf_mode=DR, skip_group_check=True,
                    )

            for c in range(NPRE):
                emit_win(c)
            for c in range(NCH):
                if c % 2 == 1:
                    emit_cast(c // 2)
                if c + NPRE < NCH:
                    emit_win(c + NPRE)
                if c % 2 == 1:
                    emit_gram(c // 2)

            nc.vector.tensor_copy(s_out.ap(), G_ps[:])

    # Fire-and-forget output DMA (walrus requires sync info on DGE ops, so
    # give it a completion semaphore nothing waits on).  The HBM write
    # receipt overlaps the runtime teardown.
    ff_sem = nc.alloc_semaphore("ff_out")
    nc.sync.dma_start(out=out_d[:], in_=s_out.ap(),
                      single_packet=True).then_inc(ff_sem, 16)

    nc.compile()
    return nc


def get_nc():
    if "nc" not in _NC_CACHE:
        _NC_CACHE["nc"] = _build_nc()
    return _NC_CACHE["nc"]


def host_prep(seq_batch, M, Acoeff, Bbasis):
    """Build per-core device inputs + host-side exact terms."""
    # seq image: [128, NCH, BPC, m] with img[p, c, j] = seq[4k+j, 128c+p]
    g = np.asarray(seq_batch, np.float32).astype(FP8)  # [B, S, m]
    imgs = np.ascontiguousarray(
        g.reshape(NCORES, BPC, NCH, 128, m).transpose(0, 3, 2, 1, 4)
    ).reshape(NCORES, 128, NCH * CW)

    # DoubleRow banded window weights: out window w (0..127) contracts
    # k-tile i, row r where 128i + r - w in [0, 8).
    r = np.arange(128)[:, None]
    w = np.arange(128)[None, :]
    wk0 = (((r - w) >= 0) & ((r - w) < RANK)).astype(np.float32) / RANK
    wk1 = (((128 + r - w) >= 0) & ((128 + r - w) < RANK)).astype(np.float32) / RANK
    wtail = wk0 * (w < TAILW)
    wmat = np.concatenate([wk0, wk1, wtail], axis=1).astype(FP8)  # [128, 384]

    full = np.concatenate(
        [np.broadcast_to(wmat, (NCORES, 128, WCOLS)), imgs], axis=2)
    full = np.ascontiguousarray(full)

    # linear terms in float64 on host: t2 = <seqsum, P>, t3 = B*||Nk||^2
    M64 = np.asarray(M, np.float64)
    kmod = np.arange(Nw) % L
    Nk = (np.asarray(Acoeff, np.float64).T[kmod]
          * np.asarray(Bbasis, np.float64)[kmod])  # [Nw, m]
    Ntil = Nk @ M64  # [Nw, m]
    csum = np.concatenate([np.zeros((1, m)), np.cumsum(Ntil, axis=0)])
    s = np.arange(S)
    lo = np.maximum(s - (RANK - 1), 0)
    hi = np.minimum(s, Nw - 1)
    P = (csum[hi + 1] - csum[lo]) / RANK  # [S, m]

    seqsum = np.asarray(seq_batch, np.float64).sum(axis=0)  # [S, m]
    t2 = float((seqsum * P).sum())
    t3 = B * float((Nk ** 2).sum())
    MtM = M64.T @ M64
    return full, MtM, t2, t3


def combine(results, MtM, t2, t3):
    """results: list of 8 arrays [128, 128] f32 (per-core G) -> scalar D."""
    G = np.zeros((m, m), np.float64)
    for r in results:
        G += np.asarray(r, np.float64)
    t1 = float((MtM * G).sum())
    D = (t1 - 2.0 * t2 + t3) / (B * Nw * m)
    return np.float32(D)


def kernel(seq_batch, M, Acoeff, Bbasis):
    from concourse.bass_utils import run_bass_kernel_spmd

    seq_batch = np.asarray(seq_batch, np.float32)
    full, MtM, t2, t3 = host_prep(seq_batch, M, Acoeff, Bbasis)

    nc = get_nc()
    in_maps = [{"seq": full[c]} for c in range(NCORES)]
    res = run_bass_kernel_spmd(nc, in_maps, core_ids=list(range(NCORES)))
    outs = [res.results[c]["out"] for c in range(NCORES)]
    return combine(outs, MtM, t2, t3)
